# revision 32
# baseline (speedup 1.0000x reference)
"""Boundary-loss kernel for Trainium2 (8 NeuronCores, pure data parallel).

Computes mean(phi_G * sigmoid(predictions)) where phi_G is the per-sample
normalized signed Euclidean distance transform (EDT) of the target mask.

Algorithm (exact, per core = one batch sample):
  1. 1D distance along W per row via log-shift min-add (bf16).
  2. Clamp to V, square.
  3. PE-transpose, windowed parabola pass along H:
       d2 = min_k (g2 shifted by k) + k^2,  k in [-(V-1), V-1].
  4. phi = sqrt(d2_out) - sqrt(d2_in); sum(phi*sigmoid(pred)) and max(d2)
     reduced on device; per-sample normalization and mean on host.

Exactness certificate: the device returns max(d2) per sample. If
max(d2) <= (V-1)^2 the windowed result provably equals the full EDT for
any input; otherwise the kernel is rebuilt with a larger window V and
rerun (value-specialized JIT; not triggered for typical random masks).
"""

import numpy as np
from contextlib import ExitStack

import concourse.bass as bass
import concourse.tile as tile
from concourse import bacc, mybir, masks
from concourse.bass_utils import run_bass_kernel_spmd

B, C, H, W = 8, 1, 256, 256
P = 128
NCHUNK = H // P          # 2 row chunks
DBIG = 300.0             # "no feature" 1D distance marker
PADV = 25000.0           # parabola pad value

Alu = mybir.AluOpType
Act = mybir.ActivationFunctionType
F32 = mybir.dt.float32
BF16 = mybir.dt.bfloat16
I32 = mybir.dt.int32

# V ladder: bf16 exact while V^2 + (V-1)^2 <= 256 (V <= 11).
_V_LADDER = [4, 8, 11, 16, 32, 64, 128, 256]


def _kernel_body(ctx: ExitStack, tc, out_ap, tgt_ap, pred_ap, V: int):
    nc = tc.nc
    use_bf16 = V <= 11
    dt_w = BF16 if use_bf16 else F32

    # 1D log-shift parameters
    shifts = []
    s = 1
    while sum(shifts) < V - 1 and s <= 128:
        shifts.append(s)
        s *= 2
    S1 = shifts[-1]               # side pad
    G1 = max(sum(shifts), S1)     # gap between segments
    SEG1 = W + G1
    # per-chunk 1D tile: [S1 | o | G1 | i | S1]
    CWO, CWI = S1, S1 + SEG1
    L1 = 2 * S1 + 2 * SEG1

    # parabola tile: [K | o_w0 | 2K | o_w1 | 2K | i_w0 | 2K | i_w1 | tail]
    K = min(V - 1, 255)
    SEG2 = 256 + 2 * K
    FP = [K + j * SEG2 for j in range(4)]           # o_w0,o_w1,i_w0,i_w1
    TW2 = K + 4 * SEG2
    SPAN = slice(K, K + 3 * SEG2 + 256)

    pool = ctx.enter_context(tc.tile_pool(name="work", bufs=1))
    tmp_pool = ctx.enter_context(tc.tile_pool(name="tmp", bufs=2))
    psum = ctx.enter_context(tc.tile_pool(name="psum", bufs=1, space="PSUM"))

    def seg3(ap_tile, start, nseg, seg, width=W):
        """[p, nseg, width] view of segments with uniform stride `seg`."""
        return (ap_tile[:, start:start + nseg * seg]
                .rearrange("p (s t) -> p s t", s=nseg)[:, :, 0:width])

    # ---- load inputs (natural layout: partition=row within chunk)
    # targets via HWDGE (sync), predictions via SWDGE (gpsimd) in parallel.
    t_i32 = pool.tile([P, NCHUNK * W], I32, tag="t")
    pred_t = pool.tile([P, NCHUNK * W], F32, tag="pred")
    nc.sync.dma_start(t_i32[:, 0:W], tgt_ap[0:P, :])
    nc.gpsimd.dma_start(t_i32[:, W:2 * W], tgt_ap[P:2 * P, :])
    nc.sync.dma_start(
        pred_t[:].rearrange("p (c w) -> p c w", c=NCHUNK),
        pred_ap.rearrange("(c p) w -> p c w", p=P))

    # ---- hoist the Copy ACT-table load off the critical path (the Sigmoid
    # and Sqrt tables are preloaded later, in function-usage order, to avoid
    # table thrash).
    dummy = pool.tile([1, 8], F32, tag="dummy")
    nc.gpsimd.memset(dummy[:], 1.0)
    dummy2 = pool.tile([1, 8], F32, tag="dummy2")
    nc.scalar.activation(dummy2[:], dummy[:], Act.Copy, bias=0.0, scale=1.0)

    # ---- per-chunk 1D pipeline: each row chunk starts as soon as its DMA
    # lands. d0_o = DBIG*(1-t) via ACT linear map (targets are exactly 0/1),
    # d0_i = DBIG - d0_o on DVE, then log-shift min-adds.
    Dc = []
    for c in range(NCHUNK):
        D = pool.tile([P, L1], dt_w, tag=f"D{c}", name=f"D{c}")
        nc.gpsimd.memset(D[:], DBIG)
        nc.scalar.activation(D[:, CWO:CWO + W], t_i32[:, c * W:(c + 1) * W],
                             Act.Copy, bias=DBIG, scale=-DBIG)
        nc.scalar.activation(D[:, CWI:CWI + W], t_i32[:, c * W:(c + 1) * W],
                             Act.Copy, bias=0.0, scale=DBIG)
        for s in shifts:
            q = tmp_pool.tile([P, L1], dt_w, tag="q1d")
            nc.vector.tensor_scalar_add(q[:], D[:], float(s))
            cc = tmp_pool.tile([P, L1], dt_w, tag="c1d")
            nc.vector.tensor_tensor(cc[:, s:L1 - s], q[:, 0:L1 - 2 * s],
                                    q[:, 2 * s:L1], op=Alu.min)
            nc.vector.tensor_tensor(D[:, s:L1 - s], D[:, s:L1 - s],
                                    cc[:, s:L1 - s], op=Alu.min)
        Dc.append(D)

    # ---- transpose g2 blocks -> gT (partition=col within w, free=(seg, row))
    ident = pool.tile([P, P], dt_w, tag="ident")
    masks.make_identity(nc, ident[:])
    identf = pool.tile([P, P], F32, tag="identf")
    masks.make_identity(nc, identf[:])

    # ---- transpose pred on the (idle) PE early; sigmoid in transposed layout
    predT_ps = psum.tile([P, NCHUNK * W], F32, tag="predT_ps")
    for w in range(NCHUNK):
        for r in range(NCHUNK):
            src = pred_t[:, r * W + w * P: r * W + (w + 1) * P]
            dst = predT_ps[:, w * W + r * P: w * W + (r + 1) * P]
            nc.tensor.matmul(dst, src, identf[:], start=True, stop=True)
    probsT = pool.tile([P, NCHUNK * W], F32, tag="probsT")
    nc.scalar.activation(probsT[:], predT_ps[:], Act.Sigmoid)
    # preload the Square table (used by the p2 assembly) after the sigmoid
    nc.scalar.square(dummy2[:], dummy[:])

    # ---- transpose the 1D distances (squaring happens on the way out of
    # PSUM during p2 assembly)
    gT = psum.tile([P, 4 * P * NCHUNK], dt_w, tag="gT")
    for tensor in range(2):          # o, i
        cwt = CWO if tensor == 0 else CWI
        for r in range(NCHUNK):
            for w in range(NCHUNK):
                src = Dc[r][:, cwt + w * P: cwt + (w + 1) * P]
                dst = gT[:, (2 * tensor + w) * 256 + r * P:
                         (2 * tensor + w) * 256 + (r + 1) * P]
                if use_bf16:
                    nc.tensor.transpose(dst, src, ident[:])
                else:
                    nc.tensor.matmul(dst, src, identf[:], start=True, stop=True)

    # ---- assemble padded parabola tile: pads via Pool, centers squared
    # out of PSUM in one strided ACT op
    p2 = pool.tile([P, TW2], dt_w, tag="p2")
    nc.gpsimd.memset(p2[:], PADV)
    nc.scalar.activation(seg3(p2, FP[0], 4, SEG2, 256),
                         gT[:].rearrange("p (s t) -> p s t", s=4), Act.Square)

    # ---- windowed parabola along H: E = min(p2, min_k (p2 <<>> k) + k^2)
    cks = []
    for k in range(1, K + 1):
        qk = tmp_pool.tile([P, TW2], dt_w, tag="qk", name=f"qk{k}")
        nc.vector.tensor_scalar_add(qk[:], p2[:], float(k * k))
        ck = pool.tile([P, TW2], dt_w, tag=f"ck{k}", name=f"ck{k}")
        nc.vector.tensor_tensor(ck[:, k:TW2 - k], qk[:, 0:TW2 - 2 * k],
                                qk[:, 2 * k:TW2], op=Alu.min)
        cks.append(ck)
    # tree-min into E
    E = pool.tile([P, TW2], dt_w, tag="E")
    nc.vector.tensor_tensor(E[:, SPAN], p2[:, SPAN], cks[0][:, SPAN],
                            op=Alu.min)
    rest = cks[1:]
    while rest:
        if len(rest) >= 2:
            a, b = rest[0], rest[1]
            m = pool.tile([P, TW2], dt_w, tag="treem", name="treem")
            nc.vector.tensor_tensor(m[:, SPAN], a[:, SPAN], b[:, SPAN],
                                    op=Alu.min)
            rest = [m] + rest[2:]
        else:
            nc.vector.tensor_tensor(E[:, SPAN], E[:, SPAN], rest[0][:, SPAN],
                                    op=Alu.min)
            rest = []
    # if K was even number of cks handled above; when len(cks)==1 nothing more
    # (E already includes cks[0]); when len(cks)>=2 the loop merged the rest.

    # ---- max(d2) over all 4 segments (denominator + exactness certificate)
    # runs on DVE in parallel with the ACT sqrts
    amax = pool.tile([P, 1], F32, tag="amax")
    nc.vector.tensor_reduce(amax[:], seg3(E, K, 4, SEG2, 256),
                            axis=mybir.AxisListType.XY, op=Alu.max)

    # ---- sum(phi * probs) = sum(sqrt_o * p) - sum(sqrt_i * p): the sqrt is
    # split per tensor and each product accumulates independently, so the
    # first product overlaps the second sqrt on ACT.
    S = pool.tile([P, TW2], F32, tag="S")
    HALF = K + 2 * SEG2
    probs3 = probsT[:].rearrange("p (s t) -> p s t", s=2)
    ssum_oi = []
    for which, fo in (("o", K), ("i", HALF)):
        nc.scalar.sqrt(S[:, fo:fo + SEG2 + 256], E[:, fo:fo + SEG2 + 256])
        acc = pool.tile([P, 1], F32, tag=f"ss{which}", name=f"ss{which}")
        prodj = tmp_pool.tile([P, NCHUNK * W], F32, tag="prodj")
        nc.vector.scalar_tensor_tensor(
            prodj[:].rearrange("p (s t) -> p s t", s=2),
            seg3(S, fo, 2, SEG2, 256), 0.0, probs3,
            op0=Alu.bypass, op1=Alu.mult, accum_out=acc[:])
        ssum_oi.append(acc)
    ssum = pool.tile([P, 1], F32, tag="ssum")
    nc.vector.tensor_tensor(ssum[:], ssum_oi[0][:], ssum_oi[1][:],
                            op=Alu.subtract)

    # ---- cross-partition reduce on PE and pack [sum, maxd2]
    onescol = pool.tile([P, 1], F32, tag="onescol")
    nc.gpsimd.memset(onescol[:], 1.0)
    ssum_ps = psum.tile([1, 1], F32, tag="ssum_ps")
    nc.tensor.matmul(ssum_ps[:], ssum[:], onescol[:], start=True, stop=True)
    amax_ps = psum.tile([1, P], F32, tag="amax_ps")
    nc.tensor.matmul(amax_ps[:], amax[:], identf[:], start=True, stop=True)

    out_t = pool.tile([1, 2], F32, tag="out")
    nc.vector.tensor_copy(out_t[:, 0:1], ssum_ps[:])
    nc.vector.reduce_max(out_t[:, 1:2], amax_ps[:], axis=mybir.AxisListType.X)
    nc.sync.dma_start(out_ap, out_t[:])


def build(V: int) -> bass.Bass:
    nc = bacc.Bacc("TRN2", target_bir_lowering=False, debug=False,
                   enable_asserts=False, num_devices=B)
    tgt_d = nc.dram_tensor("targets", [H, W], I32, kind="ExternalInput")
    pred_d = nc.dram_tensor("predictions", [H, W], F32, kind="ExternalInput")
    out_d = nc.dram_tensor("out", [1, 2], F32, kind="ExternalOutput")
    with tile.TileContext(nc) as tc:
        with ExitStack() as ctx:
            _kernel_body(ctx, tc, out_d.ap(), tgt_d.ap(), pred_d.ap(), V)
    nc.compile()
    return nc


_nc_cache: dict[int, bass.Bass] = {}
LAST_V = 4


def _run(predictions: np.ndarray, targets: np.ndarray, V: int, trace=False):
    if V not in _nc_cache:
        _nc_cache[V] = build(V)
    nc = _nc_cache[V]
    in_maps = [
        {
            "targets": np.ascontiguousarray(targets[b, 0]),
            "predictions": np.ascontiguousarray(predictions[b, 0]),
        }
        for b in range(B)
    ]
    res = run_bass_kernel_spmd(nc, in_maps, core_ids=list(range(B)), trace=trace)
    outs = np.stack([r["out"][0] for r in res.results])  # (B, 2)
    return outs, res


def kernel(predictions: np.ndarray, targets: np.ndarray) -> np.ndarray:
    global LAST_V
    predictions = np.asarray(predictions, dtype=np.float32)
    targets = np.asarray(targets, dtype=np.int32)

    fg = targets[:, 0] != 0
    nfg = fg.reshape(B, -1).sum(axis=1)
    has_fg = nfg > 0
    mixed = (nfg > 0) & (nfg < H * W)   # samples subject to the certificate

    vi = 0
    while True:
        V = _V_LADDER[vi]
        outs, _ = _run(predictions, targets, V)
        maxd2 = outs[:, 1]
        if V >= 256 or not mixed.any() or maxd2[mixed].max() <= (V - 1) ** 2:
            break
        need = np.sqrt(float(maxd2[mixed].max())) + 1
        vi += 1
        while vi < len(_V_LADDER) - 1 and (_V_LADDER[vi] - 1) < need:
            vi += 1
    LAST_V = V

    s = outs[:, 0].astype(np.float32)
    denom = np.sqrt(maxd2).astype(np.float32) + np.float32(1e-8)
    contrib = np.where(has_fg, s / denom, np.float32(0.0)).astype(np.float32)
    total = contrib.sum(dtype=np.float32) / np.float32(B * C * H * W)
    return np.float32(total)


if __name__ == "__main__":
    pred = np.load("/tmp/pred.npy")
    tgt = np.load("/tmp/tgt.npy")
    val = kernel(predictions=pred, targets=tgt)
    print("kernel loss:", repr(val))
